# revision 5
# baseline (speedup 1.0000x reference)
"""Trainium2 Bass kernel for CrossAttentionPlus.

Math (reference):
    q,k,v = proj(query,key,value); scores = q@k^T * D**-0.5
    scores = where(causal, -1e9, scores) + attention_mask
    local = softmax(scores); attn = 0.5*local + 0.5*supplied
    attn = attn / (attn.sum(-1) + 1e-9); attn = where(causal, 0, attn)
    out = (attn @ v) @ Wo + bo

Sharding: 8 cores; core c handles batch b=c//2 and heads [8*(c%2), 8*(c%2)+8).
Each core returns a partial output [Q, DIM]; host sums the two head-half
partials per batch and adds bo.

Device algorithm (per core), all matmuls in float32r (full-rate fp32):
    - Projections consume host-transposed activations (x^T: [DIM, tok]) so
      Q^T [c,q], K^T [c,q] and V [k,c] come out of the PE in natural layout.
    - Attention runs in transposed layout S^T [k, q] so that exp(S^T) and
      supplied^T are direct moving operands for the attn@V matmuls, with V as
      the stationary operand; a ones-column appended to V accumulates
      E_q = sum_k exp along the way.
    - Normalization constants: denominator sum uses sum(local)==1 exactly plus
      the host-computed full-row sum of supplied (c2 = 0.5/denom); supplied^T
      arrives pre-scaled by c2 and causally zeroed, so the device only needs
      c1 = c2/E for the exp branch: reciprocal + row-scale + gpsimd partition
      broadcast + 2 DVE ops per [64, 512] tile.
    - Causal structure: fully-masked (k>q) tiles are skipped entirely; the
      matmul column windows are 256-aligned so every f32r matmul keeps N>=256
      (full PE rate); the <=128 junk columns of odd k-blocks are memset to 0
      and diagonal tiles are masked with a triangular [128,128] tile.
    - Output projection contracts this core's 512 head-dims: out_partial
      [q, DIM] in natural layout, DMA'd straight out.
"""

import numpy as np
from contextlib import ExitStack

B, Q, KLEN, DIM, H, D = 4, 1024, 1024, 1024, 16, 64
SCALE = float(D) ** -0.5
MIX = 0.5
NEG = -1.0e9
N_CORES = 8
NH = 8            # heads per core
P = 128
NKB = KLEN // P   # 8 k-blocks
QCH = 512         # q chunk (one PSUM bank of fp32)

_BUILD_CACHE = {}


def _build(causal: bool):
    """Build + compile the Bass program. causal=True: standard causal mask;
    causal=False: no masking at all."""
    import concourse.tile as tile
    import concourse.mybir as mybir
    from concourse import bacc

    F32 = mybir.dt.float32
    F32R = mybir.dt.float32r
    AF = mybir.ActivationFunctionType
    OP = mybir.AluOpType

    nc = bacc.Bacc("TRN2", target_bir_lowering=False, debug=False,
                   num_devices=N_CORES)

    qT = nc.dram_tensor("qT", [DIM, Q], F32, kind="ExternalInput").ap()
    kT = nc.dram_tensor("kT", [DIM, KLEN], F32, kind="ExternalInput").ap()
    vT = nc.dram_tensor("vT", [DIM, KLEN], F32, kind="ExternalInput").ap()
    wq = nc.dram_tensor("wq", [DIM, NH * D], F32, kind="ExternalInput").ap()
    wk = nc.dram_tensor("wk", [DIM, NH * D], F32, kind="ExternalInput").ap()
    wv = nc.dram_tensor("wv", [DIM, NH * D], F32, kind="ExternalInput").ap()
    wo = nc.dram_tensor("wo", [NH * D, DIM], F32, kind="ExternalInput").ap()
    sup = nc.dram_tensor("sup", [NH, KLEN, Q], F32, kind="ExternalInput").ap()
    c2 = nc.dram_tensor("c2", [NH, Q], F32, kind="ExternalInput").ap()
    mT = nc.dram_tensor("mT", [P, P], F32, kind="ExternalInput").ap()
    out = nc.dram_tensor("out_p", [Q, DIM], F32, kind="ExternalOutput").ap()

    def wlo_of(kb, qc):
        # 256-aligned start column of k-block kb's unmasked window, relative
        # to chunk qc.  (Columns q < 128*kb are causally masked.)
        if not causal:
            return 0
        return max(256 * (kb // 2) - qc * QCH, 0)

    with tile.TileContext(nc) as tc:
        with ExitStack() as ctx:
            # --- pools ---
            xT_pool = ctx.enter_context(tc.tile_pool(name="xT", bufs=2))
            w_pool = ctx.enter_context(tc.tile_pool(name="w", bufs=2))
            st_pool = ctx.enter_context(tc.tile_pool(name="store", bufs=1))
            sup_pool = ctx.enter_context(tc.tile_pool(name="sup", bufs=2))
            exp_pool = ctx.enter_context(tc.tile_pool(name="exp", bufs=3))
            row_pool = ctx.enter_context(tc.tile_pool(name="rows", bufs=2))
            rep_pool = ctx.enter_context(tc.tile_pool(name="rep", bufs=2))
            tmp_pool = ctx.enter_context(tc.tile_pool(name="tmp", bufs=2))
            const_pool = ctx.enter_context(tc.tile_pool(name="const", bufs=1))
            outb_pool = ctx.enter_context(tc.tile_pool(name="outb", bufs=2))

            s_psum = ctx.enter_context(
                tc.tile_pool(name="spsum", bufs=2, space="PSUM"))
            a_psum = ctx.enter_context(
                tc.tile_pool(name="apsum", bufs=2, space="PSUM"))
            b_psum = ctx.enter_context(
                tc.tile_pool(name="bpsum", bufs=2, space="PSUM"))
            p_psum = ctx.enter_context(
                tc.tile_pool(name="ppsum", bufs=2, space="PSUM"))

            # --- constants ---
            if causal:
                mT_sb = const_pool.tile([P, P], F32R, tag="mT")
                nc.sync.dma_start(mT_sb[:], mT.bitcast(F32R))

            # --- persistent stores ---
            # QT_st/KT_st tile j holds projected heads 2j,2j+1: [c=128, q=1024]
            QT_st = [st_pool.tile([P, Q], F32R, tag=f"qt{j}", name=f"qt{j}") for j in range(4)]
            KT_st = [st_pool.tile([P, Q], F32R, tag=f"kt{j}", name=f"kt{j}") for j in range(4)]
            # V_st[kb]: [k=128, NH*(D+1)]  (per head: D cols of V then a ones col)
            V_st = [st_pool.tile([P, NH * (D + 1)], F32R, tag=f"vst{kb}", name=f"vst{kb}")
                    for kb in range(NKB)]
            # attnT tile j: [hd=128 (heads 2j,2j+1), q=1024]
            AT_st = [st_pool.tile([P, Q], F32R, tag=f"at{j}", name=f"at{j}") for j in range(4)]

            # ========== Phase 1: projections ==========
            # Q^T/K^T:  out[c, q] += W[i, c]^T-as-lhsT @ x^T[i, q]
            for name, w_ap, x_ap, dst in (
                ("q", wq, qT, QT_st), ("k", wk, kT, KT_st)):
                w_sb = w_pool.tile([P, NKB, NH * D], F32R, tag="w")
                nc.sync.dma_start(
                    w_sb[:], w_ap.bitcast(F32R).rearrange("(n p) c -> p n c", p=P))
                xh = []
                for half in range(2):
                    xt = xT_pool.tile([P, 4, Q], F32R, tag="xT")
                    nc.sync.dma_start(
                        xt[:],
                        x_ap.bitcast(F32R).rearrange(
                            "(n p) q -> p n q", p=P)[:, 4 * half:4 * half + 4, :])
                    xh.append(xt)
                for ct in range(4):
                    for qc in range(2):
                        ps = p_psum.tile([P, QCH], F32, tag="proj")
                        for ib in range(NKB):
                            nc.tensor.matmul(
                                ps[:],
                                w_sb[:, ib, ct * P:(ct + 1) * P],
                                xh[ib // 4][:, ib % 4, qc * QCH:(qc + 1) * QCH],
                                start=(ib == 0), stop=(ib == NKB - 1))
                        nc.scalar.copy(dst[ct][:, qc * QCH:(qc + 1) * QCH], ps[:])

            # V: out[k, c] += v^T[i, k]-as-lhsT @ Wv[i, c]
            wv_sb = w_pool.tile([P, NKB, NH * D], F32R, tag="w")
            nc.sync.dma_start(
                wv_sb[:], wv.bitcast(F32R).rearrange("(n p) c -> p n c", p=P))
            vh = []
            for half in range(2):
                xt = xT_pool.tile([P, 4, KLEN], F32R, tag="xT")
                nc.sync.dma_start(
                    xt[:],
                    vT.bitcast(F32R).rearrange(
                        "(n p) q -> p n q", p=P)[:, 4 * half:4 * half + 4, :])
                vh.append(xt)
            for kb in range(NKB):
                ps = p_psum.tile([P, NH * D], F32, tag="proj")
                for ib in range(NKB):
                    nc.tensor.matmul(
                        ps[:],
                        vh[ib // 4][:, ib % 4, kb * P:(kb + 1) * P],
                        wv_sb[:, ib, :],
                        start=(ib == 0), stop=(ib == NKB - 1))
                # scatter per-head 64-col groups into the 65-stride layout
                nc.scalar.copy(
                    V_st[kb][:].rearrange("p (h x) -> p h x", x=D + 1)[:, :, 0:D],
                    ps[:].rearrange("p (h x) -> p h x", x=D))
                nc.vector.memset(
                    V_st[kb][:].rearrange("p (h x) -> p h x", x=D + 1)[:, :, D:D + 1].bitcast(F32),
                    1.0)

            # ========== Phase 2: attention (per local head) ==========
            for h in range(NH):
                j, po = h // 2, (h % 2) * D
                for qc in range(2):
                    kmax = (4 * qc + 4) if causal else NKB
                    cols = slice(qc * QCH, (qc + 1) * QCH)
                    # supplied^T load: k-blocks [0, kmax) for this chunk
                    sup_t = sup_pool.tile([P, kmax, QCH], F32R, tag="sup")
                    sup_r = sup.bitcast(F32R).rearrange(
                        "h (n p) q -> h p n q", p=P)
                    if causal and qc == 0:
                        nc.sync.dma_start(
                            sup_t[:, 0:2, :], sup_r[h, :, 0:2, cols])
                        nc.sync.dma_start(
                            sup_t[:, 2:4, 256:QCH],
                            sup_r[h, :, 2:4, 256:QCH])
                    else:
                        nc.sync.dma_start(
                            sup_t[:, 0:kmax, :], sup_r[h, :, 0:kmax, cols])

                    o2a = a_psum.tile([D + 1, QCH], F32, tag="o2a")
                    o2b = b_psum.tile([D, QCH], F32, tag="o2b")
                    for kb in range(kmax):
                        wlo = wlo_of(kb, qc)
                        s_ps = s_psum.tile([P, QCH], F32, tag="s")
                        nc.tensor.matmul(
                            s_ps[:, wlo:],
                            KT_st[j][po:po + D, kb * P:(kb + 1) * P],
                            QT_st[j][po:po + D, qc * QCH + wlo:(qc + 1) * QCH],
                            start=True, stop=True)
                        e_t = exp_pool.tile([P, QCH], F32R, tag="e")
                        nc.scalar.activation(
                            e_t[:, wlo:], s_ps[:, wlo:], AF.Exp,
                            bias=0.0, scale=SCALE)
                        if causal:
                            dstart = kb * P - qc * QCH  # diag col in this chunk
                            if kb % 2 == 1 and dstart > wlo:
                                # junk columns [wlo, dstart) of odd k-blocks
                                nc.vector.memset(e_t[:, wlo:dstart].bitcast(F32), 0.0)
                            if 4 * qc <= kb < 4 * qc + 4:
                                nc.vector.tensor_tensor(
                                    out=e_t[:, dstart:dstart + P],
                                    in0=e_t[:, dstart:dstart + P],
                                    in1=mT_sb[:], op=OP.mult)
                        nc.tensor.matmul(
                            o2a[:, wlo:],
                            V_st[kb][:, h * (D + 1):(h + 1) * (D + 1)],
                            e_t[:, wlo:],
                            start=(kb == 0), stop=(kb == kmax - 1))
                        nc.tensor.matmul(
                            o2b[:, wlo:],
                            V_st[kb][:, h * (D + 1):h * (D + 1) + D],
                            sup_t[:, kb, wlo:],
                            start=(kb == 0), stop=(kb == kmax - 1))

                    # c1 = c2 / E ; attn^T = c1 (x) o2a[0:D] + o2b
                    c2row = row_pool.tile([1, QCH], F32, tag="c2row")
                    nc.sync.dma_start(c2row[:], c2[h:h + 1, cols])
                    erec = row_pool.tile([1, QCH], F32, tag="erec")
                    nc.vector.reciprocal(erec[:], o2a[D:D + 1, :])
                    c1r = row_pool.tile([1, QCH], F32, tag="c1r")
                    nc.vector.tensor_tensor(
                        out=c1r[:], in0=erec[:], in1=c2row[:],
                        op=OP.mult)
                    rep = rep_pool.tile([D, QCH], F32, tag="rep")
                    nc.gpsimd.partition_broadcast(rep[:], c1r[:])
                    t1 = tmp_pool.tile([D, QCH], F32, tag="t1")
                    nc.vector.tensor_tensor(
                        out=t1[:], in0=o2a[0:D, :], in1=rep[:], op=OP.mult)
                    nc.vector.tensor_tensor(
                        out=AT_st[j][po:po + D, cols], in0=t1[:],
                        in1=o2b[:], op=OP.add)

            # ========== Phase 3: output projection ==========
            wo_sb = w_pool.tile([P, 4, DIM], F32R, tag="w")
            nc.sync.dma_start(
                wo_sb[:], wo.bitcast(F32R).rearrange("(n p) o -> p n o", p=P))
            for m in range(8):
                for oc in range(2):
                    ps = p_psum.tile([P, QCH], F32, tag="proj")
                    for j in range(4):
                        nc.tensor.matmul(
                            ps[:],
                            AT_st[j][:, m * P:(m + 1) * P],
                            wo_sb[:, j, oc * QCH:(oc + 1) * QCH],
                            start=(j == 0), stop=(j == 3))
                    ob = outb_pool.tile([P, QCH], F32, tag="ob")
                    nc.scalar.copy(ob[:], ps[:])
                    nc.sync.dma_start(
                        out[m * P:(m + 1) * P, oc * QCH:(oc + 1) * QCH], ob[:])

    nc.compile()
    return nc


def _prep_inputs(query, key, value, supplied_attn, Wq, Wk, Wv, Wo, causal):
    """Host-side marshaling: per-core transposed slices + normalization rows."""
    f32 = np.float32
    # c2 = MIX / (MIX*sum(local) + (1-MIX)*sum(supplied) + 1e-9); sum(local)=1
    s_row = supplied_attn.sum(axis=-1, dtype=np.float32)          # [B,H,Q]
    denom = (MIX + (1.0 - MIX) * s_row + 1e-9).astype(f32)
    c2f = (np.float32(1.0 - MIX) / denom).astype(f32)             # [B,H,Q]
    c2_exp = (np.float32(MIX) / denom).astype(f32)                # scale for exp branch

    mTf = np.triu(np.ones((P, P), dtype=f32))                     # 1 where k<=q

    in_maps = []
    for core in range(N_CORES):
        b, hh = core // 2, core % 2
        h0 = hh * NH
        qTa = np.ascontiguousarray(query[b].T, dtype=f32)
        kTa = np.ascontiguousarray(key[b].T, dtype=f32)
        vTa = np.ascontiguousarray(value[b].T, dtype=f32)
        wqa = np.ascontiguousarray(Wq[:, h0 * D:(h0 + NH) * D], dtype=f32)
        wka = np.ascontiguousarray(Wk[:, h0 * D:(h0 + NH) * D], dtype=f32)
        wva = np.ascontiguousarray(Wv[:, h0 * D:(h0 + NH) * D], dtype=f32)
        woa = np.ascontiguousarray(Wo[h0 * D:(h0 + NH) * D, :], dtype=f32)
        s = supplied_attn[b, h0:h0 + NH]                          # [NH, Q, K]
        s = s * c2f[b, h0:h0 + NH, :, None]                       # pre-scale rows
        if causal:
            s = np.tril(s)                                        # zero k>q
        supa = np.ascontiguousarray(s.transpose(0, 2, 1), dtype=f32)  # [NH,K,Q]
        in_maps.append({
            "qT": qTa, "kT": kTa, "vT": vTa,
            "wq": wqa, "wk": wka, "wv": wva, "wo": woa,
            "sup": supa,
            "c2": np.ascontiguousarray(c2_exp[b, h0:h0 + NH], dtype=f32),
            "mT": mTf,
        })
    return in_maps


def _fallback_numpy(query, key, value, attention_mask, supplied_attn,
                    Wq, Wk, Wv, Wo, bo, causal_mask):
    q = (query @ Wq).reshape(B, Q, H, D).transpose(0, 2, 1, 3)
    k = (key @ Wk).reshape(B, KLEN, H, D).transpose(0, 2, 1, 3)
    v = (value @ Wv).reshape(B, KLEN, H, D).transpose(0, 2, 1, 3)
    scores = np.einsum("bhqd,bhkd->bhqk", q, k).astype(np.float32) * np.float32(SCALE)
    cm = np.broadcast_to(causal_mask, scores.shape)
    scores = np.where(cm, np.float32(NEG), scores)
    scores = scores + attention_mask
    m = scores.max(axis=-1, keepdims=True)
    e = np.exp(scores - m)
    local = e / e.sum(axis=-1, keepdims=True)
    attn = np.float32(MIX) * local + np.float32(1.0 - MIX) * supplied_attn
    attn = attn / (attn.sum(axis=-1, keepdims=True) + np.float32(1e-9))
    attn = np.where(cm, np.float32(0.0), attn)
    o = np.einsum("bhqk,bhkd->bhqd", attn, v)
    o = o.transpose(0, 2, 1, 3).reshape(B, Q, H * D)
    return (o @ Wo + bo).astype(np.float32)


def kernel(query, key, value, attention_mask, supplied_attn,
           Wq, Wk, Wv, Wo, bo, causal_mask, _collect=None):
    query = np.asarray(query); key = np.asarray(key); value = np.asarray(value)
    attention_mask = np.asarray(attention_mask)
    supplied_attn = np.asarray(supplied_attn)
    Wq = np.asarray(Wq); Wk = np.asarray(Wk); Wv = np.asarray(Wv)
    Wo = np.asarray(Wo); bo = np.asarray(bo)
    causal_mask = np.asarray(causal_mask)

    cm2 = causal_mask.reshape(causal_mask.shape[-2], causal_mask.shape[-1])
    is_std_causal = bool(
        np.array_equal(cm2, np.triu(np.ones((Q, KLEN), dtype=bool), 1)))
    is_no_mask = not causal_mask.any()
    if attention_mask.any() or not (is_std_causal or is_no_mask):
        return _fallback_numpy(query, key, value, attention_mask,
                               supplied_attn, Wq, Wk, Wv, Wo, bo, causal_mask)

    import concourse.bass_utils as bass_utils
    causal = is_std_causal
    key_ = ("causal" if causal else "nomask")
    if key_ not in _BUILD_CACHE:
        _BUILD_CACHE[key_] = _build(causal)
    nc = _BUILD_CACHE[key_]

    in_maps = _prep_inputs(query, key, value, supplied_attn, Wq, Wk, Wv, Wo,
                           causal)
    run_kwargs = dict(_collect) if _collect else {}
    res = bass_utils.run_bass_kernel_spmd(
        nc, in_maps, core_ids=list(range(N_CORES)), **run_kwargs)
    if _collect is not None:
        _collect["results"] = res

    out = np.empty((B, Q, DIM), dtype=np.float32)
    for b in range(B):
        out[b] = (res.results[2 * b]["out_p"] + res.results[2 * b + 1]["out_p"]
                  + bo.astype(np.float32))
    return out


# revision 6
# speedup vs baseline: 1.1698x; 1.1698x over previous
"""Trainium2 Bass kernel for CrossAttentionPlus.

Math (reference):
    q,k,v = proj(query,key,value); scores = q@k^T * D**-0.5
    scores = where(causal, -1e9, scores) + attention_mask
    local = softmax(scores); attn = 0.5*local + 0.5*supplied
    attn = attn / (attn.sum(-1) + 1e-9); attn = where(causal, 0, attn)
    out = (attn @ v) @ Wo + bo

Sharding: 8 cores; core c handles batch b=c//2 and heads [8*(c%2), 8*(c%2)+8).
Each core returns a partial output [Q, DIM]; host sums the two head-half
partials per batch and adds bo.

Device algorithm (per core), all matmuls in float32r (full-rate fp32):
    - Projections consume host-transposed activations (x^T: [DIM, tok]) so
      Q^T [c,q], K^T [c,q] and V [k,c] come out of the PE in natural layout.
    - Attention runs in transposed layout S^T [k, q] so that exp(S^T) and
      supplied^T are direct moving operands for the attn@V matmuls, with V as
      the stationary operand; a ones-column appended to V accumulates
      E_q = sum_k exp along the way.
    - Normalization constants: denominator sum uses sum(local)==1 exactly plus
      the host-computed full-row sum of supplied (c2 = 0.5/denom); supplied^T
      arrives pre-scaled by c2 and causally zeroed, so the device only needs
      c1 = c2/E for the exp branch: reciprocal + row-scale + gpsimd partition
      broadcast + 2 DVE ops per [64, 512] tile.
    - Causal structure: fully-masked (k>q) tiles are skipped entirely; the
      matmul column windows are 256-aligned so every f32r matmul keeps N>=256
      (full PE rate); the <=128 junk columns of odd k-blocks are memset to 0
      and diagonal tiles are masked with a triangular [128,128] tile.
    - Output projection contracts this core's 512 head-dims: out_partial
      [q, DIM] in natural layout, DMA'd straight out.
"""

import numpy as np
from contextlib import ExitStack

B, Q, KLEN, DIM, H, D = 4, 1024, 1024, 1024, 16, 64
SCALE = float(D) ** -0.5
MIX = 0.5
NEG = -1.0e9
N_CORES = 8
NH = 8            # heads per core
P = 128
NKB = KLEN // P   # 8 k-blocks
QCH = 512         # q chunk (one PSUM bank of fp32)

_BUILD_CACHE = {}


def _build(causal: bool):
    """Build + compile the Bass program. causal=True: standard causal mask;
    causal=False: no masking at all."""
    import concourse.tile as tile
    import concourse.mybir as mybir
    from concourse import bacc

    F32 = mybir.dt.float32
    F16 = mybir.dt.float16
    AF = mybir.ActivationFunctionType
    OP = mybir.AluOpType

    nc = bacc.Bacc("TRN2", target_bir_lowering=False, debug=False,
                   num_devices=N_CORES)

    qT = nc.dram_tensor("qT", [DIM, Q], F16, kind="ExternalInput").ap()
    kT = nc.dram_tensor("kT", [DIM, KLEN], F16, kind="ExternalInput").ap()
    vT = nc.dram_tensor("vT", [DIM, KLEN], F16, kind="ExternalInput").ap()
    wq = nc.dram_tensor("wq", [DIM, NH * D], F16, kind="ExternalInput").ap()
    wk = nc.dram_tensor("wk", [DIM, NH * D], F16, kind="ExternalInput").ap()
    wv = nc.dram_tensor("wv", [DIM, NH * D], F16, kind="ExternalInput").ap()
    wo = nc.dram_tensor("wo", [NH * D, DIM], F16, kind="ExternalInput").ap()
    sup = nc.dram_tensor("sup", [NH, KLEN, Q], F16, kind="ExternalInput").ap()
    c2 = nc.dram_tensor("c2", [NH, Q], F32, kind="ExternalInput").ap()
    mT = nc.dram_tensor("mT", [P, P], F16, kind="ExternalInput").ap()
    out = nc.dram_tensor("out_p", [Q, DIM], F32, kind="ExternalOutput").ap()

    def wlo_of(kb, qc):
        # 256-aligned start column of k-block kb's unmasked window, relative
        # to chunk qc.  (Columns q < 128*kb are causally masked.)
        if not causal:
            return 0
        return max(256 * (kb // 2) - qc * QCH, 0)

    with tile.TileContext(nc) as tc:
        with ExitStack() as ctx:
            # --- pools ---
            xT_pool = ctx.enter_context(tc.tile_pool(name="xT", bufs=3))
            w_pool = ctx.enter_context(tc.tile_pool(name="w", bufs=2))
            st_pool = ctx.enter_context(tc.tile_pool(name="store", bufs=1))
            sup_pool = ctx.enter_context(tc.tile_pool(name="sup", bufs=3))
            exp_pool = ctx.enter_context(tc.tile_pool(name="exp", bufs=4))
            row_pool = ctx.enter_context(tc.tile_pool(name="rows", bufs=2))
            rep_pool = ctx.enter_context(tc.tile_pool(name="rep", bufs=2))
            tmp_pool = ctx.enter_context(tc.tile_pool(name="tmp", bufs=2))
            const_pool = ctx.enter_context(tc.tile_pool(name="const", bufs=1))
            outb_pool = ctx.enter_context(tc.tile_pool(name="outb", bufs=2))

            s_psum = ctx.enter_context(
                tc.tile_pool(name="spsum", bufs=2, space="PSUM"))
            a_psum = ctx.enter_context(
                tc.tile_pool(name="apsum", bufs=2, space="PSUM"))
            b_psum = ctx.enter_context(
                tc.tile_pool(name="bpsum", bufs=2, space="PSUM"))
            p_psum = ctx.enter_context(
                tc.tile_pool(name="ppsum", bufs=2, space="PSUM"))

            # --- constants ---
            if causal:
                mT_sb = const_pool.tile([P, P], F16, tag="mT")
                nc.sync.dma_start(mT_sb[:], mT)

            # --- persistent stores ---
            # QT_st/KT_st tile j holds projected heads 2j,2j+1: [c=128, q=1024]
            QT_st = [st_pool.tile([P, Q], F16, tag=f"qt{j}", name=f"qt{j}") for j in range(4)]
            KT_st = [st_pool.tile([P, Q], F16, tag=f"kt{j}", name=f"kt{j}") for j in range(4)]
            # V_st[kb]: [k=128, NH*(D+1)]  (per head: D cols of V then a ones col)
            V_st = [st_pool.tile([P, NH * (D + 1)], F16, tag=f"vst{kb}", name=f"vst{kb}")
                    for kb in range(NKB)]
            # attnT tile j: [hd=128 (heads 2j,2j+1), q=1024]
            AT_st = [st_pool.tile([P, Q], F16, tag=f"at{j}", name=f"at{j}") for j in range(4)]

            # ========== Phase 1: projections ==========
            # Q^T/K^T:  out[c, q] += W[i, c]^T-as-lhsT @ x^T[i, q]
            for name, w_ap, x_ap, dst in (
                ("q", wq, qT, QT_st), ("k", wk, kT, KT_st)):
                w_sb = w_pool.tile([P, NKB, NH * D], F16, tag="w")
                nc.sync.dma_start(
                    w_sb[:], w_ap.rearrange("(n p) c -> p n c", p=P))
                xh = []
                for half in range(2):
                    xt = xT_pool.tile([P, 4, Q], F16, tag="xT")
                    nc.sync.dma_start(
                        xt[:],
                        x_ap.rearrange(
                            "(n p) q -> p n q", p=P)[:, 4 * half:4 * half + 4, :])
                    xh.append(xt)
                for ct in range(4):
                    for qc in range(2):
                        ps = p_psum.tile([P, QCH], F32, tag="proj")
                        for ib in range(NKB):
                            nc.tensor.matmul(
                                ps[:],
                                w_sb[:, ib, ct * P:(ct + 1) * P],
                                xh[ib // 4][:, ib % 4, qc * QCH:(qc + 1) * QCH],
                                start=(ib == 0), stop=(ib == NKB - 1))
                        nc.scalar.copy(dst[ct][:, qc * QCH:(qc + 1) * QCH], ps[:])

            # V: out[k, c] += v^T[i, k]-as-lhsT @ Wv[i, c]
            wv_sb = w_pool.tile([P, NKB, NH * D], F16, tag="w")
            nc.sync.dma_start(
                wv_sb[:], wv.rearrange("(n p) c -> p n c", p=P))
            vh = []
            for half in range(2):
                xt = xT_pool.tile([P, 4, KLEN], F16, tag="xT")
                nc.sync.dma_start(
                    xt[:],
                    vT.rearrange(
                        "(n p) q -> p n q", p=P)[:, 4 * half:4 * half + 4, :])
                vh.append(xt)
            for kb in range(NKB):
                ps = p_psum.tile([P, NH * D], F32, tag="proj")
                for ib in range(NKB):
                    nc.tensor.matmul(
                        ps[:],
                        vh[ib // 4][:, ib % 4, kb * P:(kb + 1) * P],
                        wv_sb[:, ib, :],
                        start=(ib == 0), stop=(ib == NKB - 1))
                # scatter per-head 64-col groups into the 65-stride layout
                nc.scalar.copy(
                    V_st[kb][:].rearrange("p (h x) -> p h x", x=D + 1)[:, :, 0:D],
                    ps[:].rearrange("p (h x) -> p h x", x=D))
                nc.vector.memset(
                    V_st[kb][:].rearrange("p (h x) -> p h x", x=D + 1)[:, :, D:D + 1],
                    1.0)

            # ========== Phase 2: attention (per local head) ==========
            for h in range(NH):
                j, po = h // 2, (h % 2) * D
                for qc in range(2):
                    kmax = (4 * qc + 4) if causal else NKB
                    cols = slice(qc * QCH, (qc + 1) * QCH)
                    # supplied^T load: k-blocks [0, kmax) for this chunk
                    sup_t = sup_pool.tile([P, kmax, QCH], F16, tag="sup")
                    sup_r = sup.rearrange(
                        "h (n p) q -> h p n q", p=P)
                    if causal and qc == 0:
                        nc.sync.dma_start(
                            sup_t[:, 0:2, :], sup_r[h, :, 0:2, cols])
                        nc.sync.dma_start(
                            sup_t[:, 2:4, 256:QCH],
                            sup_r[h, :, 2:4, 256:QCH])
                    else:
                        nc.sync.dma_start(
                            sup_t[:, 0:kmax, :], sup_r[h, :, 0:kmax, cols])

                    o2a = a_psum.tile([D + 1, QCH], F32, tag="o2a")
                    o2b = b_psum.tile([D, QCH], F32, tag="o2b")
                    for kb in range(kmax):
                        wlo = wlo_of(kb, qc)
                        s_ps = s_psum.tile([P, QCH], F32, tag="s")
                        nc.tensor.matmul(
                            s_ps[:, wlo:],
                            KT_st[j][po:po + D, kb * P:(kb + 1) * P],
                            QT_st[j][po:po + D, qc * QCH + wlo:(qc + 1) * QCH],
                            start=True, stop=True)
                        e_t = exp_pool.tile([P, QCH], F16, tag="e")
                        nc.scalar.activation(
                            e_t[:, wlo:], s_ps[:, wlo:], AF.Exp,
                            bias=0.0, scale=SCALE)
                        if causal:
                            dstart = kb * P - qc * QCH  # diag col in this chunk
                            if kb % 2 == 1 and dstart > wlo:
                                # junk columns [wlo, dstart) of odd k-blocks
                                nc.vector.memset(e_t[:, wlo:dstart], 0.0)
                            if 4 * qc <= kb < 4 * qc + 4:
                                nc.vector.tensor_tensor(
                                    out=e_t[:, dstart:dstart + P],
                                    in0=e_t[:, dstart:dstart + P],
                                    in1=mT_sb[:], op=OP.mult)
                        nc.tensor.matmul(
                            o2a[:, wlo:],
                            V_st[kb][:, h * (D + 1):(h + 1) * (D + 1)],
                            e_t[:, wlo:],
                            start=(kb == 0), stop=(kb == kmax - 1))
                        nc.tensor.matmul(
                            o2b[:, wlo:],
                            V_st[kb][:, h * (D + 1):h * (D + 1) + D],
                            sup_t[:, kb, wlo:],
                            start=(kb == 0), stop=(kb == kmax - 1))

                    # c1 = c2 / E ; attn^T = c1 (x) o2a[0:D] + o2b
                    c2row = row_pool.tile([1, QCH], F32, tag="c2row")
                    nc.sync.dma_start(c2row[:], c2[h:h + 1, cols])
                    erec = row_pool.tile([1, QCH], F32, tag="erec")
                    nc.vector.reciprocal(erec[:], o2a[D:D + 1, :])
                    c1r = row_pool.tile([1, QCH], F32, tag="c1r")
                    nc.vector.tensor_tensor(
                        out=c1r[:], in0=erec[:], in1=c2row[:],
                        op=OP.mult)
                    rep = rep_pool.tile([D, QCH], F32, tag="rep")
                    nc.gpsimd.partition_broadcast(rep[:], c1r[:])
                    t1 = tmp_pool.tile([D, QCH], F32, tag="t1")
                    nc.vector.tensor_tensor(
                        out=t1[:], in0=o2a[0:D, :], in1=rep[:], op=OP.mult)
                    nc.vector.tensor_tensor(
                        out=AT_st[j][po:po + D, cols], in0=t1[:],
                        in1=o2b[:], op=OP.add)

            # ========== Phase 3: output projection ==========
            wo_sb = w_pool.tile([P, 4, DIM], F16, tag="w")
            nc.sync.dma_start(
                wo_sb[:], wo.rearrange("(n p) o -> p n o", p=P))
            for m in range(8):
                for oc in range(2):
                    ps = p_psum.tile([P, QCH], F32, tag="proj")
                    for j in range(4):
                        nc.tensor.matmul(
                            ps[:],
                            AT_st[j][:, m * P:(m + 1) * P],
                            wo_sb[:, j, oc * QCH:(oc + 1) * QCH],
                            start=(j == 0), stop=(j == 3))
                    ob = outb_pool.tile([P, QCH], F32, tag="ob")
                    nc.scalar.copy(ob[:], ps[:])
                    nc.sync.dma_start(
                        out[m * P:(m + 1) * P, oc * QCH:(oc + 1) * QCH], ob[:])

    nc.compile()
    return nc


def _prep_inputs(query, key, value, supplied_attn, Wq, Wk, Wv, Wo, causal):
    """Host-side marshaling: per-core transposed slices + normalization rows."""
    f32 = np.float32
    f16 = np.float16
    # c2 = MIX / (MIX*sum(local) + (1-MIX)*sum(supplied) + 1e-9); sum(local)=1
    s_row = supplied_attn.sum(axis=-1, dtype=np.float32)          # [B,H,Q]
    denom = (MIX + (1.0 - MIX) * s_row + 1e-9).astype(f32)
    c2f = (np.float32(1.0 - MIX) / denom).astype(f32)             # [B,H,Q]
    c2_exp = (np.float32(MIX) / denom).astype(f32)                # scale for exp branch

    mTf = np.triu(np.ones((P, P), dtype=f16))                     # 1 where k<=q

    in_maps = []
    for core in range(N_CORES):
        b, hh = core // 2, core % 2
        h0 = hh * NH
        qTa = np.ascontiguousarray(query[b].T.astype(f16))
        kTa = np.ascontiguousarray(key[b].T.astype(f16))
        vTa = np.ascontiguousarray(value[b].T.astype(f16))
        wqa = np.ascontiguousarray(Wq[:, h0 * D:(h0 + NH) * D].astype(f16))
        wka = np.ascontiguousarray(Wk[:, h0 * D:(h0 + NH) * D].astype(f16))
        wva = np.ascontiguousarray(Wv[:, h0 * D:(h0 + NH) * D].astype(f16))
        woa = np.ascontiguousarray(Wo[h0 * D:(h0 + NH) * D, :].astype(f16))
        s = supplied_attn[b, h0:h0 + NH]                          # [NH, Q, K]
        s = s * c2f[b, h0:h0 + NH, :, None]                       # pre-scale rows
        if causal:
            s = np.tril(s)                                        # zero k>q
        supa = np.ascontiguousarray(s.transpose(0, 2, 1).astype(f16))  # [NH,K,Q]
        in_maps.append({
            "qT": qTa, "kT": kTa, "vT": vTa,
            "wq": wqa, "wk": wka, "wv": wva, "wo": woa,
            "sup": supa,
            "c2": np.ascontiguousarray(c2_exp[b, h0:h0 + NH], dtype=f32),
            "mT": mTf,
        })
    return in_maps


def _fallback_numpy(query, key, value, attention_mask, supplied_attn,
                    Wq, Wk, Wv, Wo, bo, causal_mask):
    q = (query @ Wq).reshape(B, Q, H, D).transpose(0, 2, 1, 3)
    k = (key @ Wk).reshape(B, KLEN, H, D).transpose(0, 2, 1, 3)
    v = (value @ Wv).reshape(B, KLEN, H, D).transpose(0, 2, 1, 3)
    scores = np.einsum("bhqd,bhkd->bhqk", q, k).astype(np.float32) * np.float32(SCALE)
    cm = np.broadcast_to(causal_mask, scores.shape)
    scores = np.where(cm, np.float32(NEG), scores)
    scores = scores + attention_mask
    m = scores.max(axis=-1, keepdims=True)
    e = np.exp(scores - m)
    local = e / e.sum(axis=-1, keepdims=True)
    attn = np.float32(MIX) * local + np.float32(1.0 - MIX) * supplied_attn
    attn = attn / (attn.sum(axis=-1, keepdims=True) + np.float32(1e-9))
    attn = np.where(cm, np.float32(0.0), attn)
    o = np.einsum("bhqk,bhkd->bhqd", attn, v)
    o = o.transpose(0, 2, 1, 3).reshape(B, Q, H * D)
    return (o @ Wo + bo).astype(np.float32)


def kernel(query, key, value, attention_mask, supplied_attn,
           Wq, Wk, Wv, Wo, bo, causal_mask, _collect=None):
    query = np.asarray(query); key = np.asarray(key); value = np.asarray(value)
    attention_mask = np.asarray(attention_mask)
    supplied_attn = np.asarray(supplied_attn)
    Wq = np.asarray(Wq); Wk = np.asarray(Wk); Wv = np.asarray(Wv)
    Wo = np.asarray(Wo); bo = np.asarray(bo)
    causal_mask = np.asarray(causal_mask)

    cm2 = causal_mask.reshape(causal_mask.shape[-2], causal_mask.shape[-1])
    is_std_causal = bool(
        np.array_equal(cm2, np.triu(np.ones((Q, KLEN), dtype=bool), 1)))
    is_no_mask = not causal_mask.any()
    if attention_mask.any() or not (is_std_causal or is_no_mask):
        return _fallback_numpy(query, key, value, attention_mask,
                               supplied_attn, Wq, Wk, Wv, Wo, bo, causal_mask)

    import concourse.bass_utils as bass_utils
    causal = is_std_causal
    key_ = ("causal" if causal else "nomask")
    if key_ not in _BUILD_CACHE:
        _BUILD_CACHE[key_] = _build(causal)
    nc = _BUILD_CACHE[key_]

    in_maps = _prep_inputs(query, key, value, supplied_attn, Wq, Wk, Wv, Wo,
                           causal)
    run_kwargs = dict(_collect) if _collect else {}
    res = bass_utils.run_bass_kernel_spmd(
        nc, in_maps, core_ids=list(range(N_CORES)), **run_kwargs)
    if _collect is not None:
        _collect["results"] = res

    out = np.empty((B, Q, DIM), dtype=np.float32)
    for b in range(B):
        out[b] = (res.results[2 * b]["out_p"] + res.results[2 * b + 1]["out_p"]
                  + bo.astype(np.float32))
    return out


# revision 10
# speedup vs baseline: 1.4976x; 1.2803x over previous
"""Trainium2 Bass kernel for CrossAttentionPlus.

Math (reference):
    q,k,v = proj(query,key,value); scores = q@k^T * D**-0.5
    scores = where(causal, -1e9, scores) + attention_mask
    local = softmax(scores); attn = 0.5*local + 0.5*supplied
    attn = attn / (attn.sum(-1) + 1e-9); attn = where(causal, 0, attn)
    out = (attn @ v) @ Wo + bo

Sharding: 8 cores; core c handles batch b=c//2 and heads [8*(c%2), 8*(c%2)+8).
Each core returns a partial output [Q, DIM]; host sums the two head-half
partials per batch and adds bo.

Device algorithm (per core), all matmuls in float32r (full-rate fp32):
    - Projections consume host-transposed activations (x^T: [DIM, tok]) so
      Q^T [c,q], K^T [c,q] and V [k,c] come out of the PE in natural layout.
    - Attention runs in transposed layout S^T [k, q] so that exp(S^T) and
      supplied^T are direct moving operands for the attn@V matmuls, with V as
      the stationary operand; a ones-column appended to V accumulates
      E_q = sum_k exp along the way.
    - Normalization constants: denominator sum uses sum(local)==1 exactly plus
      the host-computed full-row sum of supplied (c2 = 0.5/denom); supplied^T
      arrives pre-scaled by c2 and causally zeroed, so the device only needs
      c1 = c2/E for the exp branch: reciprocal + row-scale + gpsimd partition
      broadcast + 2 DVE ops per [64, 512] tile.
    - Causal structure: fully-masked (k>q) tiles are skipped entirely; the
      matmul column windows are 256-aligned so every f32r matmul keeps N>=256
      (full PE rate); the <=128 junk columns of odd k-blocks are memset to 0
      and diagonal tiles are masked with a triangular [128,128] tile.
    - Output projection contracts this core's 512 head-dims: out_partial
      [q, DIM] in natural layout, DMA'd straight out.
"""

import numpy as np
from contextlib import ExitStack

B, Q, KLEN, DIM, H, D = 4, 1024, 1024, 1024, 16, 64
SCALE = float(D) ** -0.5
MIX = 0.5
NEG = -1.0e9
N_CORES = 8
NH = 8            # heads per core
P = 128
NKB = KLEN // P   # 8 k-blocks
QCH = 512         # q chunk (one PSUM bank of fp32)

_BUILD_CACHE = {}


def _build(causal: bool):
    """Build + compile the Bass program. causal=True: standard causal mask;
    causal=False: no masking at all."""
    import concourse.tile as tile
    import concourse.mybir as mybir
    from concourse import bacc

    F32 = mybir.dt.float32
    F16 = mybir.dt.float16
    AF = mybir.ActivationFunctionType
    OP = mybir.AluOpType

    nc = bacc.Bacc("TRN2", target_bir_lowering=False, debug=False,
                   num_devices=N_CORES)

    qT = nc.dram_tensor("qT", [DIM, Q], F16, kind="ExternalInput").ap()
    kT = nc.dram_tensor("kT", [DIM, KLEN], F16, kind="ExternalInput").ap()
    vT = nc.dram_tensor("vT", [DIM, KLEN], F16, kind="ExternalInput").ap()
    wq = nc.dram_tensor("wq", [DIM, NH * D], F16, kind="ExternalInput").ap()
    wk = nc.dram_tensor("wk", [DIM, NH * D], F16, kind="ExternalInput").ap()
    wv = nc.dram_tensor("wv", [DIM, NH * D], F16, kind="ExternalInput").ap()
    wo = nc.dram_tensor("wo", [NH * D, DIM], F16, kind="ExternalInput").ap()
    sup = nc.dram_tensor("sup", [NH, KLEN, Q], F16, kind="ExternalInput").ap()
    c2 = nc.dram_tensor("c2", [NH, Q], F32, kind="ExternalInput").ap()
    mT = nc.dram_tensor("mT", [P, P], F16, kind="ExternalInput").ap()
    out = nc.dram_tensor("out_p", [Q, DIM], F32, kind="ExternalOutput").ap()

    def wlo_of(kb, qc):
        # 256-aligned start column of k-block kb's unmasked window, relative
        # to chunk qc.  (Columns q < 128*kb are causally masked.)
        if not causal:
            return 0
        return max(256 * (kb // 2) - qc * QCH, 0)

    with tile.TileContext(nc) as tc:
        with ExitStack() as ctx:
            # --- pools ---
            xT_pool = ctx.enter_context(tc.tile_pool(name="xT", bufs=3))
            w_pool = ctx.enter_context(tc.tile_pool(name="w", bufs=2))
            st_pool = ctx.enter_context(tc.tile_pool(name="store", bufs=1))
            sup_pool = ctx.enter_context(tc.tile_pool(name="sup", bufs=3))
            exp_pool = ctx.enter_context(tc.tile_pool(name="exp", bufs=4))
            row_pool = ctx.enter_context(tc.tile_pool(name="rows", bufs=2))
            rep_pool = ctx.enter_context(tc.tile_pool(name="rep", bufs=2))
            tmp_pool = ctx.enter_context(tc.tile_pool(name="tmp", bufs=2))
            const_pool = ctx.enter_context(tc.tile_pool(name="const", bufs=1))
            outb_pool = ctx.enter_context(tc.tile_pool(name="outb", bufs=2))

            s_psum = ctx.enter_context(
                tc.tile_pool(name="spsum", bufs=3, space="PSUM"))
            a_psum = ctx.enter_context(
                tc.tile_pool(name="apsum", bufs=2, space="PSUM"))
            b_psum = ctx.enter_context(
                tc.tile_pool(name="bpsum", bufs=2, space="PSUM"))

            # --- constants ---
            if causal:
                mT_sb = const_pool.tile([P, P], F16, tag="mT")
                nc.sync.dma_start(mT_sb[:], mT)

            # --- persistent stores ---
            # QT_st/KT_st tile j holds projected heads 2j,2j+1: [c=128, q=1024]
            QT_st = [st_pool.tile([P, Q], F16, tag=f"qt{j}", name=f"qt{j}") for j in range(4)]
            KT_st = [st_pool.tile([P, Q], F16, tag=f"kt{j}", name=f"kt{j}") for j in range(4)]
            # V_st[kb]: [k=128, NH*(D+1)]  (per head: D cols of V then a ones col)
            V_st = [st_pool.tile([P, NH * (D + 1)], F16, tag=f"vst{kb}", name=f"vst{kb}")
                    for kb in range(NKB)]
            # attnT tile j: [hd=128 (heads 2j,2j+1), q=1024]
            AT_st = [st_pool.tile([P, Q], F16, tag=f"at{j}", name=f"at{j}") for j in range(4)]

            # ========== Phase 1: projections ==========
            # Q^T/K^T:  out[c, q] += W[i, c]^T-as-lhsT @ x^T[i, q]
            for name, w_ap, x_ap, dst in (
                ("q", wq, qT, QT_st), ("k", wk, kT, KT_st)):
                w_sb = w_pool.tile([P, NKB, NH * D], F16, tag="w")
                nc.sync.dma_start(
                    w_sb[:], w_ap.rearrange("(n p) c -> p n c", p=P))
                xh = []
                for half in range(2):
                    xt = xT_pool.tile([P, 4, Q], F16, tag="xT")
                    nc.sync.dma_start(
                        xt[:],
                        x_ap.rearrange(
                            "(n p) q -> p n q", p=P)[:, 4 * half:4 * half + 4, :])
                    xh.append(xt)
                for ct in range(4):
                    for qc in range(2):
                        ps = s_psum.tile([P, QCH], F32, tag="s")
                        for ib in range(NKB):
                            nc.tensor.matmul(
                                ps[:],
                                w_sb[:, ib, ct * P:(ct + 1) * P],
                                xh[ib // 4][:, ib % 4, qc * QCH:(qc + 1) * QCH],
                                start=(ib == 0), stop=(ib == NKB - 1))
                        nc.scalar.copy(dst[ct][:, qc * QCH:(qc + 1) * QCH], ps[:])

            # V: out[k, c] += v^T[i, k]-as-lhsT @ Wv[i, c]
            wv_sb = w_pool.tile([P, NKB, NH * D], F16, tag="w")
            nc.sync.dma_start(
                wv_sb[:], wv.rearrange("(n p) c -> p n c", p=P))
            vh = []
            for half in range(2):
                xt = xT_pool.tile([P, 4, KLEN], F16, tag="xT")
                nc.sync.dma_start(
                    xt[:],
                    vT.rearrange(
                        "(n p) q -> p n q", p=P)[:, 4 * half:4 * half + 4, :])
                vh.append(xt)
            for kb in range(NKB):
                ps = s_psum.tile([P, NH * D], F32, tag="s")
                for ib in range(NKB):
                    nc.tensor.matmul(
                        ps[:],
                        vh[ib // 4][:, ib % 4, kb * P:(kb + 1) * P],
                        wv_sb[:, ib, :],
                        start=(ib == 0), stop=(ib == NKB - 1))
                # scatter per-head 64-col groups into the 65-stride layout
                nc.scalar.copy(
                    V_st[kb][:].rearrange("p (h x) -> p h x", x=D + 1)[:, :, 0:D],
                    ps[:].rearrange("p (h x) -> p h x", x=D))
                nc.vector.memset(
                    V_st[kb][:].rearrange("p (h x) -> p h x", x=D + 1)[:, :, D:D + 1],
                    1.0)

            # ========== Phase 2: attention (per local head) ==========
            for h in range(NH):
                j, po = h // 2, (h % 2) * D
                for qc in range(2):
                    kmax = (4 * qc + 4) if causal else NKB
                    cols = slice(qc * QCH, (qc + 1) * QCH)
                    # supplied^T load: k-blocks [0, kmax) for this chunk
                    sup_t = sup_pool.tile([P, kmax, QCH], F16, tag="sup")
                    sup_r = sup.rearrange(
                        "h (n p) q -> h p n q", p=P)
                    if causal and qc == 0:
                        nc.sync.dma_start(
                            sup_t[:, 0:2, :], sup_r[h, :, 0:2, cols])
                        nc.sync.dma_start(
                            sup_t[:, 2:4, 256:QCH],
                            sup_r[h, :, 2:4, 256:QCH])
                    else:
                        nc.sync.dma_start(
                            sup_t[:, 0:kmax, :], sup_r[h, :, 0:kmax, cols])

                    o2a = a_psum.tile([D + 1, QCH], F32, tag="o2a")
                    o2b = b_psum.tile([D, QCH], F32, tag="o2b")
                    for kb in range(kmax):
                        wlo = wlo_of(kb, qc)
                        s_ps = s_psum.tile([P, QCH], F32, tag="s")
                        nc.tensor.matmul(
                            s_ps[:, wlo:],
                            KT_st[j][po:po + D, kb * P:(kb + 1) * P],
                            QT_st[j][po:po + D, qc * QCH + wlo:(qc + 1) * QCH],
                            start=True, stop=True)
                        e_t = exp_pool.tile([P, QCH], F16, tag="e")
                        nc.scalar.activation(
                            e_t[:, wlo:], s_ps[:, wlo:], AF.Exp,
                            bias=0.0, scale=SCALE)
                        if causal:
                            dstart = kb * P - qc * QCH  # diag col in this chunk
                            if kb % 2 == 1 and dstart > wlo:
                                # junk columns [wlo, dstart) of odd k-blocks
                                nc.vector.memset(e_t[:, wlo:dstart], 0.0)
                            if 4 * qc <= kb < 4 * qc + 4:
                                nc.vector.tensor_tensor(
                                    out=e_t[:, dstart:dstart + P],
                                    in0=e_t[:, dstart:dstart + P],
                                    in1=mT_sb[:], op=OP.mult)
                        nc.tensor.matmul(
                            o2a[:, wlo:],
                            V_st[kb][:, h * (D + 1):(h + 1) * (D + 1)],
                            e_t[:, wlo:],
                            start=(kb == 0), stop=(kb == kmax - 1))
                        nc.tensor.matmul(
                            o2b[:, wlo:],
                            V_st[kb][:, h * (D + 1):h * (D + 1) + D],
                            sup_t[:, kb, wlo:],
                            start=(kb == 0), stop=(kb == kmax - 1))

                    # c1 = c2 / E ; attn^T = c1 (x) o2a[0:D] + o2b
                    c2row = row_pool.tile([1, QCH], F32, tag="c2row")
                    nc.sync.dma_start(c2row[:], c2[h:h + 1, cols])
                    ecopy = row_pool.tile([1, QCH], F32, tag="ecopy")
                    nc.vector.tensor_copy(out=ecopy[:], in_=o2a[D:D + 1, :])
                    erec = row_pool.tile([1, QCH], F32, tag="erec")
                    nc.vector.reciprocal_approx_fast(erec[:], ecopy[:])
                    c1r = row_pool.tile([1, QCH], F32, tag="c1r")
                    nc.vector.tensor_tensor(
                        out=c1r[:], in0=erec[:], in1=c2row[:],
                        op=OP.mult)
                    rep = rep_pool.tile([D, QCH], F32, tag="rep")
                    nc.gpsimd.partition_broadcast(rep[:], c1r[:])
                    t1 = tmp_pool.tile([D, QCH], F32, tag="t1")
                    nc.vector.tensor_tensor(
                        out=t1[:], in0=o2a[0:D, :], in1=rep[:], op=OP.mult)
                    nc.vector.tensor_tensor(
                        out=AT_st[j][po:po + D, cols], in0=t1[:],
                        in1=o2b[:], op=OP.add)

            # ========== Phase 3: output projection ==========
            wo_sb = w_pool.tile([P, 4, DIM], F16, tag="w")
            nc.sync.dma_start(
                wo_sb[:], wo.rearrange("(n p) o -> p n o", p=P))
            for m in range(8):
                for oc in range(2):
                    ps = s_psum.tile([P, QCH], F32, tag="s")
                    for j in range(4):
                        nc.tensor.matmul(
                            ps[:],
                            AT_st[j][:, m * P:(m + 1) * P],
                            wo_sb[:, j, oc * QCH:(oc + 1) * QCH],
                            start=(j == 0), stop=(j == 3))
                    ob = outb_pool.tile([P, QCH], F32, tag="ob")
                    nc.scalar.copy(ob[:], ps[:])
                    nc.sync.dma_start(
                        out[m * P:(m + 1) * P, oc * QCH:(oc + 1) * QCH], ob[:])

    nc.compile()
    return nc


def _prep_inputs(query, key, value, supplied_attn, Wq, Wk, Wv, Wo, causal):
    """Host-side marshaling: per-core transposed slices + normalization rows."""
    f32 = np.float32
    f16 = np.float16
    # c2 = MIX / (MIX*sum(local) + (1-MIX)*sum(supplied) + 1e-9); sum(local)=1
    s_row = supplied_attn.sum(axis=-1, dtype=np.float32)          # [B,H,Q]
    denom = (MIX + (1.0 - MIX) * s_row + 1e-9).astype(f32)
    c2f = (np.float32(1.0 - MIX) / denom).astype(f32)             # [B,H,Q]
    c2_exp = (np.float32(MIX) / denom).astype(f32)                # scale for exp branch

    mTf = np.triu(np.ones((P, P), dtype=f16))                     # 1 where k<=q

    in_maps = []
    for core in range(N_CORES):
        b, hh = core // 2, core % 2
        h0 = hh * NH
        qTa = np.ascontiguousarray(query[b].T.astype(f16))
        kTa = np.ascontiguousarray(key[b].T.astype(f16))
        vTa = np.ascontiguousarray(value[b].T.astype(f16))
        wqa = np.ascontiguousarray(Wq[:, h0 * D:(h0 + NH) * D].astype(f16))
        wka = np.ascontiguousarray(Wk[:, h0 * D:(h0 + NH) * D].astype(f16))
        wva = np.ascontiguousarray(Wv[:, h0 * D:(h0 + NH) * D].astype(f16))
        woa = np.ascontiguousarray(Wo[h0 * D:(h0 + NH) * D, :].astype(f16))
        s = supplied_attn[b, h0:h0 + NH]                          # [NH, Q, K]
        s = s * c2f[b, h0:h0 + NH, :, None]                       # pre-scale rows
        if causal:
            s = np.tril(s)                                        # zero k>q
        supa = np.ascontiguousarray(s.transpose(0, 2, 1).astype(f16))  # [NH,K,Q]
        in_maps.append({
            "qT": qTa, "kT": kTa, "vT": vTa,
            "wq": wqa, "wk": wka, "wv": wva, "wo": woa,
            "sup": supa,
            "c2": np.ascontiguousarray(c2_exp[b, h0:h0 + NH], dtype=f32),
            "mT": mTf,
        })
    return in_maps


def _fallback_numpy(query, key, value, attention_mask, supplied_attn,
                    Wq, Wk, Wv, Wo, bo, causal_mask):
    q = (query @ Wq).reshape(B, Q, H, D).transpose(0, 2, 1, 3)
    k = (key @ Wk).reshape(B, KLEN, H, D).transpose(0, 2, 1, 3)
    v = (value @ Wv).reshape(B, KLEN, H, D).transpose(0, 2, 1, 3)
    scores = np.einsum("bhqd,bhkd->bhqk", q, k).astype(np.float32) * np.float32(SCALE)
    cm = np.broadcast_to(causal_mask, scores.shape)
    scores = np.where(cm, np.float32(NEG), scores)
    scores = scores + attention_mask
    m = scores.max(axis=-1, keepdims=True)
    e = np.exp(scores - m)
    local = e / e.sum(axis=-1, keepdims=True)
    attn = np.float32(MIX) * local + np.float32(1.0 - MIX) * supplied_attn
    attn = attn / (attn.sum(axis=-1, keepdims=True) + np.float32(1e-9))
    attn = np.where(cm, np.float32(0.0), attn)
    o = np.einsum("bhqk,bhkd->bhqd", attn, v)
    o = o.transpose(0, 2, 1, 3).reshape(B, Q, H * D)
    return (o @ Wo + bo).astype(np.float32)


def kernel(query, key, value, attention_mask, supplied_attn,
           Wq, Wk, Wv, Wo, bo, causal_mask, _collect=None):
    query = np.asarray(query); key = np.asarray(key); value = np.asarray(value)
    attention_mask = np.asarray(attention_mask)
    supplied_attn = np.asarray(supplied_attn)
    Wq = np.asarray(Wq); Wk = np.asarray(Wk); Wv = np.asarray(Wv)
    Wo = np.asarray(Wo); bo = np.asarray(bo)
    causal_mask = np.asarray(causal_mask)

    cm2 = causal_mask.reshape(causal_mask.shape[-2], causal_mask.shape[-1])
    is_std_causal = bool(
        np.array_equal(cm2, np.triu(np.ones((Q, KLEN), dtype=bool), 1)))
    is_no_mask = not causal_mask.any()
    if attention_mask.any() or not (is_std_causal or is_no_mask):
        return _fallback_numpy(query, key, value, attention_mask,
                               supplied_attn, Wq, Wk, Wv, Wo, bo, causal_mask)

    import concourse.bass_utils as bass_utils
    causal = is_std_causal
    key_ = ("causal" if causal else "nomask")
    if key_ not in _BUILD_CACHE:
        _BUILD_CACHE[key_] = _build(causal)
    nc = _BUILD_CACHE[key_]

    in_maps = _prep_inputs(query, key, value, supplied_attn, Wq, Wk, Wv, Wo,
                           causal)
    run_kwargs = dict(_collect) if _collect else {}
    res = bass_utils.run_bass_kernel_spmd(
        nc, in_maps, core_ids=list(range(N_CORES)), **run_kwargs)
    if _collect is not None:
        _collect["results"] = res

    out = np.empty((B, Q, DIM), dtype=np.float32)
    for b in range(B):
        out[b] = (res.results[2 * b]["out_p"] + res.results[2 * b + 1]["out_p"]
                  + bo.astype(np.float32))
    return out


# revision 11
# speedup vs baseline: 1.5530x; 1.0370x over previous
"""Trainium2 Bass kernel for CrossAttentionPlus.

Math (reference):
    q,k,v = proj(query,key,value); scores = q@k^T * D**-0.5
    scores = where(causal, -1e9, scores) + attention_mask
    local = softmax(scores); attn = 0.5*local + 0.5*supplied
    attn = attn / (attn.sum(-1) + 1e-9); attn = where(causal, 0, attn)
    out = (attn @ v) @ Wo + bo

Sharding: 8 cores; core c handles batch b=c//2 and heads [8*(c%2), 8*(c%2)+8).
Each core returns a partial output [Q, DIM]; host sums the two head-half
partials per batch and adds bo.

Device algorithm (per core), all matmuls in float32r (full-rate fp32):
    - Projections consume host-transposed activations (x^T: [DIM, tok]) so
      Q^T [c,q], K^T [c,q] and V [k,c] come out of the PE in natural layout.
    - Attention runs in transposed layout S^T [k, q] so that exp(S^T) and
      supplied^T are direct moving operands for the attn@V matmuls, with V as
      the stationary operand; a ones-column appended to V accumulates
      E_q = sum_k exp along the way.
    - Normalization constants: denominator sum uses sum(local)==1 exactly plus
      the host-computed full-row sum of supplied (c2 = 0.5/denom); supplied^T
      arrives pre-scaled by c2 and causally zeroed, so the device only needs
      c1 = c2/E for the exp branch: reciprocal + row-scale + gpsimd partition
      broadcast + 2 DVE ops per [64, 512] tile.
    - Causal structure: fully-masked (k>q) tiles are skipped entirely; the
      matmul column windows are 256-aligned so every f32r matmul keeps N>=256
      (full PE rate); the <=128 junk columns of odd k-blocks are memset to 0
      and diagonal tiles are masked with a triangular [128,128] tile.
    - Output projection contracts this core's 512 head-dims: out_partial
      [q, DIM] in natural layout, DMA'd straight out.
"""

import numpy as np
from contextlib import ExitStack

B, Q, KLEN, DIM, H, D = 4, 1024, 1024, 1024, 16, 64
SCALE = float(D) ** -0.5
MIX = 0.5
NEG = -1.0e9
N_CORES = 8
NH = 8            # heads per core
P = 128
NKB = KLEN // P   # 8 k-blocks
QCH = 512         # q chunk (one PSUM bank of fp32)

_BUILD_CACHE = {}


def _build(causal: bool):
    """Build + compile the Bass program. causal=True: standard causal mask;
    causal=False: no masking at all."""
    import concourse.tile as tile
    import concourse.mybir as mybir
    from concourse import bacc

    F32 = mybir.dt.float32
    F16 = mybir.dt.float16
    AF = mybir.ActivationFunctionType
    OP = mybir.AluOpType

    nc = bacc.Bacc("TRN2", target_bir_lowering=False, debug=False,
                   num_devices=N_CORES)

    qT = nc.dram_tensor("qT", [DIM, Q], F16, kind="ExternalInput").ap()
    kT = nc.dram_tensor("kT", [DIM, KLEN], F16, kind="ExternalInput").ap()
    vT = nc.dram_tensor("vT", [DIM, KLEN], F16, kind="ExternalInput").ap()
    wq = nc.dram_tensor("wq", [DIM, NH * D], F16, kind="ExternalInput").ap()
    wk = nc.dram_tensor("wk", [DIM, NH * D], F16, kind="ExternalInput").ap()
    wv = nc.dram_tensor("wv", [DIM, NH * D], F16, kind="ExternalInput").ap()
    wo = nc.dram_tensor("wo", [NH * D, DIM], F16, kind="ExternalInput").ap()
    sup = nc.dram_tensor("sup", [NH, KLEN, Q], F16, kind="ExternalInput").ap()
    c2 = nc.dram_tensor("c2", [NH, Q], F32, kind="ExternalInput").ap()
    mT = nc.dram_tensor("mT", [P, P], F16, kind="ExternalInput").ap()
    out = nc.dram_tensor("out_p", [Q, DIM], F32, kind="ExternalOutput").ap()

    def wlo_of(kb, qc):
        # 256-aligned start column of k-block kb's unmasked window, relative
        # to chunk qc.  (Columns q < 128*kb are causally masked.)
        if not causal:
            return 0
        return max(P * kb - qc * QCH, 0)

    with tile.TileContext(nc) as tc:
        with ExitStack() as ctx:
            # --- pools ---
            xT_pool = ctx.enter_context(tc.tile_pool(name="xT", bufs=3))
            w_pool = ctx.enter_context(tc.tile_pool(name="w", bufs=2))
            st_pool = ctx.enter_context(tc.tile_pool(name="store", bufs=1))
            sup_pool = ctx.enter_context(tc.tile_pool(name="sup", bufs=3))
            exp_pool = ctx.enter_context(tc.tile_pool(name="exp", bufs=4))
            row_pool = ctx.enter_context(tc.tile_pool(name="rows", bufs=2))
            rep_pool = ctx.enter_context(tc.tile_pool(name="rep", bufs=2))
            tmp_pool = ctx.enter_context(tc.tile_pool(name="tmp", bufs=2))
            const_pool = ctx.enter_context(tc.tile_pool(name="const", bufs=1))
            outb_pool = ctx.enter_context(tc.tile_pool(name="outb", bufs=2))

            s_psum = ctx.enter_context(
                tc.tile_pool(name="spsum", bufs=3, space="PSUM"))
            a_psum = ctx.enter_context(
                tc.tile_pool(name="apsum", bufs=2, space="PSUM"))
            b_psum = ctx.enter_context(
                tc.tile_pool(name="bpsum", bufs=2, space="PSUM"))

            # --- constants ---
            if causal:
                mT_sb = const_pool.tile([P, P], F16, tag="mT")
                nc.sync.dma_start(mT_sb[:], mT)

            # --- persistent stores ---
            # QT_st/KT_st tile j holds projected heads 2j,2j+1: [c=128, q=1024]
            QT_st = [st_pool.tile([P, Q], F16, tag=f"qt{j}", name=f"qt{j}") for j in range(4)]
            KT_st = [st_pool.tile([P, Q], F16, tag=f"kt{j}", name=f"kt{j}") for j in range(4)]
            # V_st[kb]: [k=128, NH*(D+1)]  (per head: D cols of V then a ones col)
            V_st = [st_pool.tile([P, NH * (D + 1)], F16, tag=f"vst{kb}", name=f"vst{kb}")
                    for kb in range(NKB)]
            # attnT tile j: [hd=128 (heads 2j,2j+1), q=1024]
            AT_st = [st_pool.tile([P, Q], F16, tag=f"at{j}", name=f"at{j}") for j in range(4)]

            # ========== Phase 1: projections ==========
            # Q^T/K^T:  out[c, q] += W[i, c]^T-as-lhsT @ x^T[i, q]
            for name, w_ap, x_ap, dst in (
                ("q", wq, qT, QT_st), ("k", wk, kT, KT_st)):
                w_sb = w_pool.tile([P, NKB, NH * D], F16, tag="w")
                w_r = w_ap.rearrange("(n p) c -> p n c", p=P)
                for ib in range(NKB):
                    nc.sync.dma_start(w_sb[:, ib:ib + 1, :], w_r[:, ib:ib + 1, :])
                xh = []
                for half in range(2):
                    xt = xT_pool.tile([P, 4, Q], F16, tag="xT")
                    x_r = x_ap.rearrange("(n p) q -> p n q", p=P)
                    for sb in range(4):
                        nc.sync.dma_start(
                            xt[:, sb:sb + 1, :],
                            x_r[:, 4 * half + sb:4 * half + sb + 1, :])
                    xh.append(xt)
                for ct in range(4):
                    for qc in range(2):
                        ps = s_psum.tile([P, QCH], F32, tag="s")
                        for ib in range(NKB):
                            nc.tensor.matmul(
                                ps[:],
                                w_sb[:, ib, ct * P:(ct + 1) * P],
                                xh[ib // 4][:, ib % 4, qc * QCH:(qc + 1) * QCH],
                                start=(ib == 0), stop=(ib == NKB - 1))
                        nc.scalar.copy(dst[ct][:, qc * QCH:(qc + 1) * QCH], ps[:])

            # V: out[k, c] += v^T[i, k]-as-lhsT @ Wv[i, c]
            wv_sb = w_pool.tile([P, NKB, NH * D], F16, tag="w")
            wv_r = wv.rearrange("(n p) c -> p n c", p=P)
            for ib in range(NKB):
                nc.sync.dma_start(wv_sb[:, ib:ib + 1, :], wv_r[:, ib:ib + 1, :])
            vh = []
            for half in range(2):
                xt = xT_pool.tile([P, 4, KLEN], F16, tag="xT")
                vT_r = vT.rearrange("(n p) q -> p n q", p=P)
                for sb in range(4):
                    nc.sync.dma_start(
                        xt[:, sb:sb + 1, :],
                        vT_r[:, 4 * half + sb:4 * half + sb + 1, :])
                vh.append(xt)
            for kb in range(NKB):
                ps = s_psum.tile([P, NH * D], F32, tag="s")
                for ib in range(NKB):
                    nc.tensor.matmul(
                        ps[:],
                        vh[ib // 4][:, ib % 4, kb * P:(kb + 1) * P],
                        wv_sb[:, ib, :],
                        start=(ib == 0), stop=(ib == NKB - 1))
                # scatter per-head 64-col groups into the 65-stride layout
                nc.scalar.copy(
                    V_st[kb][:].rearrange("p (h x) -> p h x", x=D + 1)[:, :, 0:D],
                    ps[:].rearrange("p (h x) -> p h x", x=D))
                nc.vector.memset(
                    V_st[kb][:].rearrange("p (h x) -> p h x", x=D + 1)[:, :, D:D + 1],
                    1.0)

            # ========== Phase 2: attention (per local head) ==========
            for h in range(NH):
                j, po = h // 2, (h % 2) * D
                for qc in range(2):
                    kmax = (4 * qc + 4) if causal else NKB
                    cols = slice(qc * QCH, (qc + 1) * QCH)
                    # supplied^T load: k-blocks [0, kmax) for this chunk
                    sup_t = sup_pool.tile([P, kmax, QCH], F16, tag="sup")
                    sup_r = sup.rearrange(
                        "h (n p) q -> h p n q", p=P)
                    if causal and qc == 0:
                        for kb2 in range(0, 4, 2):
                            w2 = wlo_of(kb2, 0)
                            nc.sync.dma_start(
                                sup_t[:, kb2:kb2 + 2, w2:],
                                sup_r[h, :, kb2:kb2 + 2,
                                      qc * QCH + w2:(qc + 1) * QCH])
                    else:
                        nc.sync.dma_start(
                            sup_t[:, 0:kmax, :], sup_r[h, :, 0:kmax, cols])

                    o2a = a_psum.tile([D + 1, QCH], F32, tag="o2a")
                    o2b = b_psum.tile([D, QCH], F32, tag="o2b")
                    for kb in range(kmax):
                        wlo = wlo_of(kb, qc)
                        s_ps = s_psum.tile([P, QCH], F32, tag="s")
                        nc.tensor.matmul(
                            s_ps[:, wlo:],
                            KT_st[j][po:po + D, kb * P:(kb + 1) * P],
                            QT_st[j][po:po + D, qc * QCH + wlo:(qc + 1) * QCH],
                            start=True, stop=True)
                        e_t = exp_pool.tile([P, QCH], F16, tag="e")
                        nc.scalar.activation(
                            e_t[:, wlo:], s_ps[:, wlo:], AF.Exp,
                            bias=0.0, scale=SCALE)
                        if causal:
                            dstart = kb * P - qc * QCH  # diag col in this chunk
                            if 4 * qc <= kb < 4 * qc + 4:
                                nc.vector.tensor_tensor(
                                    out=e_t[:, dstart:dstart + P],
                                    in0=e_t[:, dstart:dstart + P],
                                    in1=mT_sb[:], op=OP.mult)
                        nc.tensor.matmul(
                            o2a[:, wlo:],
                            V_st[kb][:, h * (D + 1):(h + 1) * (D + 1)],
                            e_t[:, wlo:],
                            start=(kb == 0), stop=(kb == kmax - 1))
                        nc.tensor.matmul(
                            o2b[:, wlo:],
                            V_st[kb][:, h * (D + 1):h * (D + 1) + D],
                            sup_t[:, kb, wlo:],
                            start=(kb == 0), stop=(kb == kmax - 1))

                    # c1 = c2 / E ; attn^T = c1 (x) o2a[0:D] + o2b
                    c2row = row_pool.tile([1, QCH], F32, tag="c2row")
                    nc.sync.dma_start(c2row[:], c2[h:h + 1, cols])
                    ecopy = row_pool.tile([1, QCH], F32, tag="ecopy")
                    nc.vector.tensor_copy(out=ecopy[:], in_=o2a[D:D + 1, :])
                    erec = row_pool.tile([1, QCH], F32, tag="erec")
                    nc.vector.reciprocal_approx_fast(erec[:], ecopy[:])
                    c1r = row_pool.tile([1, QCH], F32, tag="c1r")
                    nc.vector.tensor_tensor(
                        out=c1r[:], in0=erec[:], in1=c2row[:],
                        op=OP.mult)
                    rep = rep_pool.tile([D, QCH], F32, tag="rep")
                    nc.gpsimd.partition_broadcast(rep[:], c1r[:])
                    t1 = tmp_pool.tile([D, QCH], F32, tag="t1")
                    nc.vector.tensor_tensor(
                        out=t1[:], in0=o2a[0:D, :], in1=rep[:], op=OP.mult)
                    nc.vector.tensor_tensor(
                        out=AT_st[j][po:po + D, cols], in0=t1[:],
                        in1=o2b[:], op=OP.add)

            # ========== Phase 3: output projection ==========
            wo_sb = w_pool.tile([P, 4, DIM], F16, tag="w")
            nc.sync.dma_start(
                wo_sb[:], wo.rearrange("(n p) o -> p n o", p=P))
            for m in range(8):
                for oc in range(2):
                    ps = s_psum.tile([P, QCH], F32, tag="s")
                    for j in range(4):
                        nc.tensor.matmul(
                            ps[:],
                            AT_st[j][:, m * P:(m + 1) * P],
                            wo_sb[:, j, oc * QCH:(oc + 1) * QCH],
                            start=(j == 0), stop=(j == 3))
                    ob = outb_pool.tile([P, QCH], F32, tag="ob")
                    nc.scalar.copy(ob[:], ps[:])
                    nc.sync.dma_start(
                        out[m * P:(m + 1) * P, oc * QCH:(oc + 1) * QCH], ob[:])

    nc.compile()
    return nc


def _prep_inputs(query, key, value, supplied_attn, Wq, Wk, Wv, Wo, causal):
    """Host-side marshaling: per-core transposed slices + normalization rows."""
    f32 = np.float32
    f16 = np.float16
    # c2 = MIX / (MIX*sum(local) + (1-MIX)*sum(supplied) + 1e-9); sum(local)=1
    s_row = supplied_attn.sum(axis=-1, dtype=np.float32)          # [B,H,Q]
    denom = (MIX + (1.0 - MIX) * s_row + 1e-9).astype(f32)
    c2f = (np.float32(1.0 - MIX) / denom).astype(f32)             # [B,H,Q]
    c2_exp = (np.float32(MIX) / denom).astype(f32)                # scale for exp branch

    mTf = np.triu(np.ones((P, P), dtype=f16))                     # 1 where k<=q

    in_maps = []
    for core in range(N_CORES):
        b, hh = core // 2, core % 2
        h0 = hh * NH
        qTa = np.ascontiguousarray(query[b].T.astype(f16))
        kTa = np.ascontiguousarray(key[b].T.astype(f16))
        vTa = np.ascontiguousarray(value[b].T.astype(f16))
        wqa = np.ascontiguousarray(Wq[:, h0 * D:(h0 + NH) * D].astype(f16))
        wka = np.ascontiguousarray(Wk[:, h0 * D:(h0 + NH) * D].astype(f16))
        wva = np.ascontiguousarray(Wv[:, h0 * D:(h0 + NH) * D].astype(f16))
        woa = np.ascontiguousarray(Wo[h0 * D:(h0 + NH) * D, :].astype(f16))
        s = supplied_attn[b, h0:h0 + NH]                          # [NH, Q, K]
        s = s * c2f[b, h0:h0 + NH, :, None]                       # pre-scale rows
        if causal:
            s = np.tril(s)                                        # zero k>q
        supa = np.ascontiguousarray(s.transpose(0, 2, 1).astype(f16))  # [NH,K,Q]
        in_maps.append({
            "qT": qTa, "kT": kTa, "vT": vTa,
            "wq": wqa, "wk": wka, "wv": wva, "wo": woa,
            "sup": supa,
            "c2": np.ascontiguousarray(c2_exp[b, h0:h0 + NH], dtype=f32),
            "mT": mTf,
        })
    return in_maps


def _fallback_numpy(query, key, value, attention_mask, supplied_attn,
                    Wq, Wk, Wv, Wo, bo, causal_mask):
    q = (query @ Wq).reshape(B, Q, H, D).transpose(0, 2, 1, 3)
    k = (key @ Wk).reshape(B, KLEN, H, D).transpose(0, 2, 1, 3)
    v = (value @ Wv).reshape(B, KLEN, H, D).transpose(0, 2, 1, 3)
    scores = np.einsum("bhqd,bhkd->bhqk", q, k).astype(np.float32) * np.float32(SCALE)
    cm = np.broadcast_to(causal_mask, scores.shape)
    scores = np.where(cm, np.float32(NEG), scores)
    scores = scores + attention_mask
    m = scores.max(axis=-1, keepdims=True)
    e = np.exp(scores - m)
    local = e / e.sum(axis=-1, keepdims=True)
    attn = np.float32(MIX) * local + np.float32(1.0 - MIX) * supplied_attn
    attn = attn / (attn.sum(axis=-1, keepdims=True) + np.float32(1e-9))
    attn = np.where(cm, np.float32(0.0), attn)
    o = np.einsum("bhqk,bhkd->bhqd", attn, v)
    o = o.transpose(0, 2, 1, 3).reshape(B, Q, H * D)
    return (o @ Wo + bo).astype(np.float32)


def kernel(query, key, value, attention_mask, supplied_attn,
           Wq, Wk, Wv, Wo, bo, causal_mask, _collect=None):
    query = np.asarray(query); key = np.asarray(key); value = np.asarray(value)
    attention_mask = np.asarray(attention_mask)
    supplied_attn = np.asarray(supplied_attn)
    Wq = np.asarray(Wq); Wk = np.asarray(Wk); Wv = np.asarray(Wv)
    Wo = np.asarray(Wo); bo = np.asarray(bo)
    causal_mask = np.asarray(causal_mask)

    cm2 = causal_mask.reshape(causal_mask.shape[-2], causal_mask.shape[-1])
    is_std_causal = bool(
        np.array_equal(cm2, np.triu(np.ones((Q, KLEN), dtype=bool), 1)))
    is_no_mask = not causal_mask.any()
    if attention_mask.any() or not (is_std_causal or is_no_mask):
        return _fallback_numpy(query, key, value, attention_mask,
                               supplied_attn, Wq, Wk, Wv, Wo, bo, causal_mask)

    import concourse.bass_utils as bass_utils
    causal = is_std_causal
    key_ = ("causal" if causal else "nomask")
    if key_ not in _BUILD_CACHE:
        _BUILD_CACHE[key_] = _build(causal)
    nc = _BUILD_CACHE[key_]

    in_maps = _prep_inputs(query, key, value, supplied_attn, Wq, Wk, Wv, Wo,
                           causal)
    run_kwargs = dict(_collect) if _collect else {}
    res = bass_utils.run_bass_kernel_spmd(
        nc, in_maps, core_ids=list(range(N_CORES)), **run_kwargs)
    if _collect is not None:
        _collect["results"] = res

    out = np.empty((B, Q, DIM), dtype=np.float32)
    for b in range(B):
        out[b] = (res.results[2 * b]["out_p"] + res.results[2 * b + 1]["out_p"]
                  + bo.astype(np.float32))
    return out
